# revision 27
# baseline (speedup 1.0000x reference)
import numpy as np

RETINA = 224.0
NUM_CLASSES = 4
B = 8
S = 2048
M = S - 1
NCORES = 8
BIG = 1.0e13
K12 = 8
K34 = 6
CHUNK = 512
NCOLS = 16
DIAG_W = 130

_CACHE = {}
TRACE_KWARGS = {}
LAST_RESULTS = None


def _split_multi_waits(nc, max_waits=1):
    import concourse.mybir as mybir
    for fn in nc.m.functions:
        for blk in fn.blocks:
            out = []
            changed = False
            for inst in blk.instructions:
                si = inst.sync_info
                ow = list(si.on_wait) if (si is not None and si.on_wait) else []
                if len(ow) > max_waits:
                    for k, w in enumerate(ow[:-max_waits]):
                        out.append(mybir.InstNoOp(
                            name=f"{inst.name}_wsplit{k}",
                            engine=inst.engine,
                            ins=[], outs=[],
                            sync_info=mybir.SyncInfo(on_wait=[w],
                                                     on_update=[]),
                        ))
                    si.on_wait = ow[-max_waits:]
                    changed = True
                out.append(inst)
            if changed:
                blk.instructions = out
    return nc


def _features(pts):
    sx, sy = pts[:-1, 0], pts[:-1, 1]
    eX, eY = pts[1:, 0], pts[1:, 1]
    ex, ey = eX - sx, eY - sy
    c = ex * sy - ey * sx
    g0, g1, g2 = ex, -ey, -c
    one = np.ones(M)
    U6 = np.stack([g0 * g0, g1 * g1, g2 * g2,
                   g0 * g1, g0 * g2, g1 * g2], 0)
    V6 = np.stack([sy * eY, sx * eX, one,
                   sy * eX + sx * eY,
                   sy + eY,
                   sx + eX], 0)
    return U6, V6


def _host_prep(pp, ts, pm):
    import ml_dtypes
    bfd = ml_dtypes.bfloat16
    tc_cls = ts[:, :, 4].astype(np.int32)
    valid = ~pm
    nn = valid & (tc_cls != 0)

    per_core = []
    n_segs = []
    for b in range(B):
        order = np.argsort(~nn[b], kind="stable")
        pts = pp[b][order].astype(np.float64)
        n = int(nn[b].sum())
        n_seg = n - 1
        n_segs.append(n_seg)
        if n > 0:
            pts = pts - pts[:n].mean(axis=0)
        U6, V6 = _features(pts)
        one = np.ones(M)
        inv = (np.arange(M) >= max(n_seg, 0)).astype(np.float64) * BIG
        A12 = np.concatenate([V6, inv[None], one[None]], 0)
        B12 = np.concatenate([U6, one[None], inv[None]], 0)
        per_core.append(dict(n=n, n_seg=n_seg,
                             A12=A12.astype(bfd), B12=B12.astype(bfd),
                             A34=U6.astype(bfd), B34=V6.astype(bfd)))
    return tc_cls, valid, nn, per_core, n_segs


def _schedule(L, Jmax):
    fulls, tails = [], []
    for ib in range(L // 128):
        i0 = 128 * ib
        j0 = i0
        while j0 < Jmax:
            N = min(CHUNK, Jmax - j0)
            (fulls if N == CHUNK else tails).append((i0, j0, N))
            j0 += N
    order = []
    fi = ti = 0
    while fi < len(fulls) or ti < len(tails):
        if fi < len(fulls):
            order.append(fulls[fi])
            fi += 1
        if ti < len(tails):
            order.append(tails[ti])
            ti += 1
    chunks = []
    pos = 0
    for (i0, j0, N) in order:
        chunks.append((i0, j0, N, pos))
        pos += N
    ops = []
    c = 0
    while c < len(chunks):
        k = 1 if c < 2 else min(2, len(chunks) - c)
        ops.append((c, k, chunks[c][3], sum(chunks[c + t][2] for t in range(k))))
        c += k
    opidx = {}
    for oi, (c0, k, _, _) in enumerate(ops):
        for t in range(k):
            opidx[c0 + t] = oi
    assert len(ops) <= NCOLS
    return chunks, ops, opidx, pos


def _build_program(L, Jmax):
    import concourse.bass as bass
    from concourse import mybir

    f32 = mybir.dt.float32
    bf16 = mybir.dt.bfloat16
    ALU = mybir.AluOpType
    ACT = mybir.ActivationFunctionType

    chunks, ops, opidx, total = _schedule(L, Jmax)
    SW = -(-total // 16) * 16
    NG = len(ops)

    nc = bass.Bass()
    d_feat = nc.dram_tensor("feat", [K12 + K34, 2 * L], bf16,
                            kind="ExternalInput")
    d_out = nc.dram_tensor("partials", [128, NG], f32, kind="ExternalOutput")

    with (
        nc.sbuf_tensor([38, 2 * L], bf16) as feat,
        nc.sbuf_tensor([128, SW], bf16) as sring,
        nc.sbuf_tensor([128, NCOLS], f32) as cols,
        nc.psum_tensor([128, 4096], f32) as qps,
        nc.semaphore("dma12_sem") as dma12_sem,
        nc.semaphore("dma34_sem") as dma34_sem,
        nc.semaphore("strt_sem") as strt_sem,
        nc.semaphore("pe12_sem") as pe12_sem,
        nc.semaphore("pe34_sem") as pe34_sem,
        nc.semaphore("act_sem") as act_sem,
        nc.semaphore("dve_sem") as dve_sem,
        nc.semaphore("out_sem") as out_sem,
        nc.Block() as block,
    ):
        fB12 = feat[0:K12, 0:L]
        fA12 = feat[0:K12, L:2 * L]
        fB34 = feat[32:32 + K34, 0:L]
        fA34 = feat[32:32 + K34, L:2 * L]
        JW = min(-(-Jmax // 8) * 8, L)

        @block.sync
        def _(sy):
            sy.dma_start(out=feat[0:K12, 0:JW],
                         in_=d_feat[0:K12, 0:JW]).then_inc(strt_sem, 16)
            sy.dma_start(out=feat[0:K12, L + 128:2 * L],
                         in_=d_feat[0:K12, L + 128:2 * L]).then_inc(dma12_sem, 16)

        @block.scalar
        def _(act):
            act.dma_start(out=feat[0:K12, L:L + 128],
                          in_=d_feat[0:K12, L:L + 128]).then_inc(strt_sem, 16)
            nc.scalar.activation(out=sring[:, 0:8], in_=sring[:, 0:8],
                                 func=ACT.Sigmoid, scale=0.0)
            for oi, (c0, k, pos0, W) in enumerate(ops):
                act.wait_ge(pe12_sem, oi + 1)
                o12 = 512 * (c0 % 4)
                nc.scalar.activation(out=sring[:, pos0:pos0 + W],
                                     in_=qps[:, o12:o12 + W],
                                     func=ACT.Sigmoid,
                                     scale=-0.01).then_inc(act_sem, 1)
            act.wait_ge(dve_sem, NG)
            act.dma_start(out=d_out[:, 0:NG],
                          in_=cols[:, 0:NG]).then_inc(out_sem, 16)

        @block.gpsimd
        def _(gp):
            gp.dma_start(out=feat[32:32 + K34, :],
                         in_=d_feat[K12:K12 + K34, :]).then_inc(dma34_sem, 16)

        @block.tensor
        def _(pe):
            nch = len(chunks)
            events = []
            for idx in range(nch + 2):
                if idx < nch:
                    events.append(("q12", idx))
                if idx - 2 >= 0:
                    events.append(("q34", idx - 2))
            first12 = True
            first34 = True
            for kind, c in events:
                oi = opidx[c]
                c0, k = ops[oi][0], ops[oi][1]
                t = c - c0
                i0, j0, N, _ = chunks[c]
                bank = c % 4
                if kind == "q12":
                    if first12:
                        pe.wait_ge(strt_sem, 32)
                        first12 = False
                    elif t == 0:
                        if c == 2:
                            pe.wait_ge(dma12_sem, 16)
                        if c >= 4:
                            pe.wait_ge(act_sem, opidx[c - 4] + 1)
                    mm = nc.tensor.matmul(
                        qps[:, 512 * bank:512 * bank + N],
                        fA12[:, i0:i0 + 128],
                        fB12[:, j0:j0 + N],
                        start=True, stop=True, tile_position=(0, 0))
                    if t == k - 1:
                        mm.then_inc(pe12_sem, 1)
                else:
                    if first34:
                        pe.wait_ge(dma34_sem, 16)
                        first34 = False
                    elif t == 0 and c >= 4:
                        pe.wait_ge(dve_sem, opidx[c - 4] + 1)
                    mm = nc.tensor.matmul(
                        qps[:, 2048 + 512 * bank:2048 + 512 * bank + N],
                        fA34[:, i0:i0 + 128],
                        fB34[:, j0:j0 + N],
                        start=True, stop=True, tile_position=(32, 0))
                    if t == k - 1:
                        mm.then_inc(pe34_sem, 1)

        @block.vector
        def _(dve):
            for oi, (c0, k, pos0, W) in enumerate(ops):
                dve.wait_ge(act_sem, oi + 1)
                dve.wait_ge(pe34_sem, oi + 1)
                o34 = 2048 + 512 * (c0 % 4)
                nc.vector.scalar_tensor_tensor(
                    out=sring[:, pos0:pos0 + W],
                    in0=qps[:, o34:o34 + W],
                    scalar=0.0, op0=ALU.is_lt,
                    in1=sring[:, pos0:pos0 + W], op1=ALU.mult,
                    accum_out=cols[:, oi:oi + 1]).then_inc(dve_sem, 1)


    _split_multi_waits(nc)
    return nc


def _sig(x):
    with np.errstate(over="ignore"):
        return 1.0 / (1.0 + np.exp(np.clip(x, -500.0, 500.0)))


def kernel(point_pred, orient_pred, class_pred, target_seq, padding_mask):
    pp = np.ascontiguousarray(np.asarray(point_pred, dtype=np.float32))
    op = np.ascontiguousarray(np.asarray(orient_pred, dtype=np.float32))
    cp = np.ascontiguousarray(np.asarray(class_pred, dtype=np.float32))
    ts = np.ascontiguousarray(np.asarray(target_seq, dtype=np.float32))
    pm = np.ascontiguousarray(np.asarray(padding_mask)).astype(bool)

    tc_cls, valid, nn, per_core, n_segs = _host_prep(pp, ts, pm)

    nsmax = max(max(n_segs), 1)
    L = max(128, -(-nsmax // 128) * 128)
    L = min(L, -(-M // 128) * 128)
    Jmax = min(-(-nsmax // 8) * 8, L)

    key = (L, Jmax)
    if key not in _CACHE:
        _CACHE[key] = _build_program(L, Jmax)
    nc = _CACHE[key]
    chunks, ops, _opidx, _total = _schedule(L, Jmax)
    NG = len(ops)

    import ml_dtypes
    bfdt = ml_dtypes.bfloat16
    in_maps = []
    for b in range(B):
        pc = per_core[b]
        featpk = np.zeros((K12 + K34, 2 * L), bfdt)
        w = min(M, L)
        featpk[0:K12, 0:w] = pc["B12"][:, :w]
        featpk[0:K12, L:L + w] = pc["A12"][:, :w]
        featpk[K12:K12 + K34, 0:w] = pc["B34"][:, :w]
        featpk[K12:K12 + K34, L:L + w] = pc["A34"][:, :w]
        if L > M:
            big_bf = bfdt(BIG)
            featpk[6, L + M:2 * L] = big_bf
            featpk[7, L + M:2 * L] = bfdt(1.0)
            featpk[6, M:L] = bfdt(1.0)
            featpk[7, M:L] = big_bf
        in_maps.append({"feat": np.ascontiguousarray(featpk)})

    from concourse.bass_utils import run_bass_kernel_spmd
    global LAST_RESULTS
    kw = dict(TRACE_KWARGS) if TRACE_KWARGS else {}
    res = run_bass_kernel_spmd(nc, in_maps, core_ids=list(range(NCORES)), **kw)
    LAST_RESULTS = res
    parts = [r["partials"] for r in res.results]

    f32 = np.float32

    isect_sum = np.float64(0.0)
    cnt_total = 0
    nb = L // 128
    for b in range(B):
        pc = per_core[b]
        n, n_seg = pc["n"], pc["n_seg"]
        raw = np.float64(parts[b][:, :NG].astype(np.float64).sum())
        A12 = pc["A12"].astype(np.float32)
        B12 = pc["B12"].astype(np.float32)
        A34 = pc["A34"].astype(np.float32)
        B34 = pc["B34"].astype(np.float32)
        junk = np.float64(0.0)
        for ib in range(nb):
            i0 = 128 * ib
            jw = min(DIAG_W, Jmax - i0)
            if jw <= 0:
                continue
            ia, ib_ = i0, min(i0 + 128, M)
            ja, jb_ = i0, min(i0 + jw, M)
            q12 = A12[:, ia:ib_].T @ B12[:, ja:jb_]
            q34 = A34[:, ia:ib_].T @ B34[:, ja:jb_]
            di = np.arange(ib_ - ia)[:, None]
            dj = np.arange(jb_ - ja)[None, :]
            msk = dj < di + 2
            s = _sig(0.01 * q12.astype(np.float64)) * (q34 < 0.0)
            junk += np.where(msk, s, 0.0).sum()
        wrap = np.float64(0.0)
        if n >= 4:
            jw_ = n_seg - 1
            q12w = np.float64(A12[:, 0].astype(np.float64)
                              @ B12[:, jw_].astype(np.float64))
            q34w = np.float64(A34[:, 0].astype(np.float64)
                              @ B34[:, jw_].astype(np.float64))
            wrap = _sig(0.01 * q12w) * float(q34w < 0.0)
            cnt_total += (n_seg - 1) * (n_seg - 2) // 2 - 1
            isect_sum += raw - junk - wrap

    if cnt_total > 0:
        isect_loss = f32(isect_sum / cnt_total)
    else:
        isect_loss = f32(0.0)

    valid_f = valid.astype(np.float64)
    nn_f = nn.astype(np.float64)
    vden = max(valid_f.sum(), 1.0)
    nden = max(nn_f.sum(), 1.0)

    x = cp.astype(np.float64)
    xmax = x.max(axis=-1, keepdims=True)
    lse = np.log(np.exp(x - xmax).sum(axis=-1)) + xmax[..., 0]
    sel = np.take_along_axis(x, tc_cls[..., None], axis=-1)[..., 0]
    cls_loss = f32(((lse - sel) * valid_f).sum() / vden)

    d = (pp.astype(np.float64) - ts[:, :, :2].astype(np.float64)) / RETINA
    ad = np.abs(d)
    sl1 = np.where(ad < 1.0, 0.5 * d * d, ad - 0.5).mean(axis=-1)
    pt_loss = f32((sl1 * nn_f).sum() / nden)

    cos = (op.astype(np.float64) * ts[:, :, 2:4].astype(np.float64)).sum(-1)
    orient_loss = f32(((1.0 - cos) * nn_f).sum() / nden)

    total = f32(pt_loss + f32(0.5) * orient_loss + cls_loss
                + f32(0.1) * isect_loss)
    return (total, pt_loss, orient_loss, cls_loss, isect_loss)


# revision 28
# speedup vs baseline: 1.0016x; 1.0016x over previous
import numpy as np

RETINA = 224.0
NUM_CLASSES = 4
B = 8
S = 2048
M = S - 1
NCORES = 8
BIG = 1.0e13
K12 = 8
K34 = 6
CHUNK = 512
NCOLS = 16
DIAG_W = 130

_CACHE = {}
TRACE_KWARGS = {}
LAST_RESULTS = None


def _split_multi_waits(nc, max_waits=1):
    import concourse.mybir as mybir
    for fn in nc.m.functions:
        for blk in fn.blocks:
            out = []
            changed = False
            for inst in blk.instructions:
                si = inst.sync_info
                ow = list(si.on_wait) if (si is not None and si.on_wait) else []
                if len(ow) > max_waits:
                    for k, w in enumerate(ow[:-max_waits]):
                        out.append(mybir.InstNoOp(
                            name=f"{inst.name}_wsplit{k}",
                            engine=inst.engine,
                            ins=[], outs=[],
                            sync_info=mybir.SyncInfo(on_wait=[w],
                                                     on_update=[]),
                        ))
                    si.on_wait = ow[-max_waits:]
                    changed = True
                out.append(inst)
            if changed:
                blk.instructions = out
    return nc


def _features(pts):
    sx, sy = pts[:-1, 0], pts[:-1, 1]
    eX, eY = pts[1:, 0], pts[1:, 1]
    ex, ey = eX - sx, eY - sy
    c = ex * sy - ey * sx
    g0, g1, g2 = ex, -ey, -c
    one = np.ones(M)
    U6 = np.stack([g0 * g0, g1 * g1, g2 * g2,
                   g0 * g1, g0 * g2, g1 * g2], 0)
    V6 = np.stack([sy * eY, sx * eX, one,
                   sy * eX + sx * eY,
                   sy + eY,
                   sx + eX], 0)
    return U6, V6


def _host_prep(pp, ts, pm):
    import ml_dtypes
    bfd = ml_dtypes.bfloat16
    tc_cls = ts[:, :, 4].astype(np.int32)
    valid = ~pm
    nn = valid & (tc_cls != 0)

    per_core = []
    n_segs = []
    for b in range(B):
        order = np.argsort(~nn[b], kind="stable")
        pts = pp[b][order].astype(np.float64)
        n = int(nn[b].sum())
        n_seg = n - 1
        n_segs.append(n_seg)
        if n > 0:
            pts = pts - pts[:n].mean(axis=0)
        U6, V6 = _features(pts)
        one = np.ones(M)
        inv = (np.arange(M) >= max(n_seg, 0)).astype(np.float64) * BIG
        A12 = np.concatenate([V6, inv[None], one[None]], 0)
        B12 = np.concatenate([U6, one[None], inv[None]], 0)
        per_core.append(dict(n=n, n_seg=n_seg,
                             A12=A12.astype(bfd), B12=B12.astype(bfd),
                             A34=U6.astype(bfd), B34=V6.astype(bfd)))
    return tc_cls, valid, nn, per_core, n_segs


def _schedule(L, Jmax):
    fulls, tails = [], []
    for ib in range(L // 128):
        i0 = 128 * ib
        j0 = i0
        while j0 < Jmax:
            N = min(CHUNK, Jmax - j0)
            (fulls if N == CHUNK else tails).append((i0, j0, N))
            j0 += N
    if len(tails) > 1:
        tails = [tails[0]] + sorted(tails[1:], key=lambda c: -c[2])
    order = []
    fi = ti = 0
    while fi < len(fulls) or ti < len(tails):
        if fi < len(fulls):
            order.append(fulls[fi])
            fi += 1
        if ti < len(tails):
            order.append(tails[ti])
            ti += 1
    chunks = []
    pos = 0
    for (i0, j0, N) in order:
        chunks.append((i0, j0, N, pos))
        pos += N
    ops = []
    c = 0
    while c < len(chunks):
        k = 1 if c < 2 else min(2, len(chunks) - c)
        ops.append((c, k, chunks[c][3], sum(chunks[c + t][2] for t in range(k))))
        c += k
    opidx = {}
    for oi, (c0, k, _, _) in enumerate(ops):
        for t in range(k):
            opidx[c0 + t] = oi
    assert len(ops) <= NCOLS
    return chunks, ops, opidx, pos


def _build_program(L, Jmax):
    import concourse.bass as bass
    from concourse import mybir

    f32 = mybir.dt.float32
    bf16 = mybir.dt.bfloat16
    ALU = mybir.AluOpType
    ACT = mybir.ActivationFunctionType

    chunks, ops, opidx, total = _schedule(L, Jmax)
    SW = -(-total // 16) * 16
    NG = len(ops)

    nc = bass.Bass()
    d_feat = nc.dram_tensor("feat", [K12 + K34, 2 * L], bf16,
                            kind="ExternalInput")
    d_out = nc.dram_tensor("partials", [128, NG], f32, kind="ExternalOutput")

    with (
        nc.sbuf_tensor([38, 2 * L], bf16) as feat,
        nc.sbuf_tensor([128, SW], bf16) as sring,
        nc.sbuf_tensor([128, NCOLS], f32) as cols,
        nc.psum_tensor([128, 4096], f32) as qps,
        nc.semaphore("dma12_sem") as dma12_sem,
        nc.semaphore("dma34_sem") as dma34_sem,
        nc.semaphore("strt_sem") as strt_sem,
        nc.semaphore("pe12_sem") as pe12_sem,
        nc.semaphore("pe34_sem") as pe34_sem,
        nc.semaphore("act_sem") as act_sem,
        nc.semaphore("dve_sem") as dve_sem,
        nc.semaphore("out_sem") as out_sem,
        nc.Block() as block,
    ):
        fB12 = feat[0:K12, 0:L]
        fA12 = feat[0:K12, L:2 * L]
        fB34 = feat[32:32 + K34, 0:L]
        fA34 = feat[32:32 + K34, L:2 * L]
        JW = min(-(-Jmax // 8) * 8, L)

        @block.sync
        def _(sy):
            sy.dma_start(out=feat[0:K12, 0:JW],
                         in_=d_feat[0:K12, 0:JW]).then_inc(strt_sem, 16)
            sy.dma_start(out=feat[0:K12, L + 128:2 * L],
                         in_=d_feat[0:K12, L + 128:2 * L]).then_inc(dma12_sem, 16)

        @block.scalar
        def _(act):
            act.dma_start(out=feat[0:K12, L:L + 128],
                          in_=d_feat[0:K12, L:L + 128]).then_inc(strt_sem, 16)
            nc.scalar.activation(out=sring[:, 0:8], in_=sring[:, 0:8],
                                 func=ACT.Sigmoid, scale=0.0)
            for oi, (c0, k, pos0, W) in enumerate(ops):
                act.wait_ge(pe12_sem, oi + 1)
                o12 = 512 * (c0 % 4)
                nc.scalar.activation(out=sring[:, pos0:pos0 + W],
                                     in_=qps[:, o12:o12 + W],
                                     func=ACT.Sigmoid,
                                     scale=-0.01).then_inc(act_sem, 1)
            act.wait_ge(dve_sem, NG)
            act.dma_start(out=d_out[:, 0:NG],
                          in_=cols[:, 0:NG]).then_inc(out_sem, 16)

        @block.gpsimd
        def _(gp):
            gp.dma_start(out=feat[32:32 + K34, :],
                         in_=d_feat[K12:K12 + K34, :]).then_inc(dma34_sem, 16)

        @block.tensor
        def _(pe):
            nch = len(chunks)
            events = []
            for idx in range(nch + 2):
                if idx < nch:
                    events.append(("q12", idx))
                if idx - 2 >= 0:
                    events.append(("q34", idx - 2))
            first12 = True
            first34 = True
            for kind, c in events:
                oi = opidx[c]
                c0, k = ops[oi][0], ops[oi][1]
                t = c - c0
                i0, j0, N, _ = chunks[c]
                bank = c % 4
                if kind == "q12":
                    if first12:
                        pe.wait_ge(strt_sem, 32)
                        first12 = False
                    elif t == 0:
                        if c == 2:
                            pe.wait_ge(dma12_sem, 16)
                        if c >= 4:
                            pe.wait_ge(act_sem, opidx[c - 4] + 1)
                    mm = nc.tensor.matmul(
                        qps[:, 512 * bank:512 * bank + N],
                        fA12[:, i0:i0 + 128],
                        fB12[:, j0:j0 + N],
                        start=True, stop=True, tile_position=(0, 0))
                    if t == k - 1:
                        mm.then_inc(pe12_sem, 1)
                else:
                    if first34:
                        pe.wait_ge(dma34_sem, 16)
                        first34 = False
                    elif t == 0 and c >= 4:
                        pe.wait_ge(dve_sem, opidx[c - 4] + 1)
                    mm = nc.tensor.matmul(
                        qps[:, 2048 + 512 * bank:2048 + 512 * bank + N],
                        fA34[:, i0:i0 + 128],
                        fB34[:, j0:j0 + N],
                        start=True, stop=True, tile_position=(32, 0))
                    if t == k - 1:
                        mm.then_inc(pe34_sem, 1)

        @block.vector
        def _(dve):
            for oi, (c0, k, pos0, W) in enumerate(ops):
                dve.wait_ge(pe34_sem, oi + 1)
                dve.wait_ge(act_sem, oi + 1)
                o34 = 2048 + 512 * (c0 % 4)
                nc.vector.scalar_tensor_tensor(
                    out=sring[:, pos0:pos0 + W],
                    in0=qps[:, o34:o34 + W],
                    scalar=0.0, op0=ALU.is_lt,
                    in1=sring[:, pos0:pos0 + W], op1=ALU.mult,
                    accum_out=cols[:, oi:oi + 1]).then_inc(dve_sem, 1)


    _split_multi_waits(nc)
    return nc


def _sig(x):
    with np.errstate(over="ignore"):
        return 1.0 / (1.0 + np.exp(np.clip(x, -500.0, 500.0)))


def kernel(point_pred, orient_pred, class_pred, target_seq, padding_mask):
    pp = np.ascontiguousarray(np.asarray(point_pred, dtype=np.float32))
    op = np.ascontiguousarray(np.asarray(orient_pred, dtype=np.float32))
    cp = np.ascontiguousarray(np.asarray(class_pred, dtype=np.float32))
    ts = np.ascontiguousarray(np.asarray(target_seq, dtype=np.float32))
    pm = np.ascontiguousarray(np.asarray(padding_mask)).astype(bool)

    tc_cls, valid, nn, per_core, n_segs = _host_prep(pp, ts, pm)

    nsmax = max(max(n_segs), 1)
    L = max(128, -(-nsmax // 128) * 128)
    L = min(L, -(-M // 128) * 128)
    Jmax = min(-(-nsmax // 8) * 8, L)

    key = (L, Jmax)
    if key not in _CACHE:
        _CACHE[key] = _build_program(L, Jmax)
    nc = _CACHE[key]
    chunks, ops, _opidx, _total = _schedule(L, Jmax)
    NG = len(ops)

    import ml_dtypes
    bfdt = ml_dtypes.bfloat16
    in_maps = []
    for b in range(B):
        pc = per_core[b]
        featpk = np.zeros((K12 + K34, 2 * L), bfdt)
        w = min(M, L)
        featpk[0:K12, 0:w] = pc["B12"][:, :w]
        featpk[0:K12, L:L + w] = pc["A12"][:, :w]
        featpk[K12:K12 + K34, 0:w] = pc["B34"][:, :w]
        featpk[K12:K12 + K34, L:L + w] = pc["A34"][:, :w]
        if L > M:
            big_bf = bfdt(BIG)
            featpk[6, L + M:2 * L] = big_bf
            featpk[7, L + M:2 * L] = bfdt(1.0)
            featpk[6, M:L] = bfdt(1.0)
            featpk[7, M:L] = big_bf
        in_maps.append({"feat": np.ascontiguousarray(featpk)})

    from concourse.bass_utils import run_bass_kernel_spmd
    global LAST_RESULTS
    kw = dict(TRACE_KWARGS) if TRACE_KWARGS else {}
    res = run_bass_kernel_spmd(nc, in_maps, core_ids=list(range(NCORES)), **kw)
    LAST_RESULTS = res
    parts = [r["partials"] for r in res.results]

    f32 = np.float32

    isect_sum = np.float64(0.0)
    cnt_total = 0
    nb = L // 128
    for b in range(B):
        pc = per_core[b]
        n, n_seg = pc["n"], pc["n_seg"]
        raw = np.float64(parts[b][:, :NG].astype(np.float64).sum())
        A12 = pc["A12"].astype(np.float32)
        B12 = pc["B12"].astype(np.float32)
        A34 = pc["A34"].astype(np.float32)
        B34 = pc["B34"].astype(np.float32)
        junk = np.float64(0.0)
        for ib in range(nb):
            i0 = 128 * ib
            jw = min(DIAG_W, Jmax - i0)
            if jw <= 0:
                continue
            ia, ib_ = i0, min(i0 + 128, M)
            ja, jb_ = i0, min(i0 + jw, M)
            q12 = A12[:, ia:ib_].T @ B12[:, ja:jb_]
            q34 = A34[:, ia:ib_].T @ B34[:, ja:jb_]
            di = np.arange(ib_ - ia)[:, None]
            dj = np.arange(jb_ - ja)[None, :]
            msk = dj < di + 2
            s = _sig(0.01 * q12.astype(np.float64)) * (q34 < 0.0)
            junk += np.where(msk, s, 0.0).sum()
        wrap = np.float64(0.0)
        if n >= 4:
            jw_ = n_seg - 1
            q12w = np.float64(A12[:, 0].astype(np.float64)
                              @ B12[:, jw_].astype(np.float64))
            q34w = np.float64(A34[:, 0].astype(np.float64)
                              @ B34[:, jw_].astype(np.float64))
            wrap = _sig(0.01 * q12w) * float(q34w < 0.0)
            cnt_total += (n_seg - 1) * (n_seg - 2) // 2 - 1
            isect_sum += raw - junk - wrap

    if cnt_total > 0:
        isect_loss = f32(isect_sum / cnt_total)
    else:
        isect_loss = f32(0.0)

    valid_f = valid.astype(np.float64)
    nn_f = nn.astype(np.float64)
    vden = max(valid_f.sum(), 1.0)
    nden = max(nn_f.sum(), 1.0)

    x = cp.astype(np.float64)
    xmax = x.max(axis=-1, keepdims=True)
    lse = np.log(np.exp(x - xmax).sum(axis=-1)) + xmax[..., 0]
    sel = np.take_along_axis(x, tc_cls[..., None], axis=-1)[..., 0]
    cls_loss = f32(((lse - sel) * valid_f).sum() / vden)

    d = (pp.astype(np.float64) - ts[:, :, :2].astype(np.float64)) / RETINA
    ad = np.abs(d)
    sl1 = np.where(ad < 1.0, 0.5 * d * d, ad - 0.5).mean(axis=-1)
    pt_loss = f32((sl1 * nn_f).sum() / nden)

    cos = (op.astype(np.float64) * ts[:, :, 2:4].astype(np.float64)).sum(-1)
    orient_loss = f32(((1.0 - cos) * nn_f).sum() / nden)

    total = f32(pt_loss + f32(0.5) * orient_loss + cls_loss
                + f32(0.1) * isect_loss)
    return (total, pt_loss, orient_loss, cls_loss, isect_loss)


# revision 29
# speedup vs baseline: 1.0072x; 1.0057x over previous
import numpy as np

RETINA = 224.0
NUM_CLASSES = 4
B = 8
S = 2048
M = S - 1
NCORES = 8
BIG = 1.0e13
K12 = 8
K34 = 6
CHUNK = 512
NCOLS = 16
DIAG_W = 130

_CACHE = {}
TRACE_KWARGS = {}
LAST_RESULTS = None


def _split_multi_waits(nc, max_waits=1):
    import concourse.mybir as mybir
    for fn in nc.m.functions:
        for blk in fn.blocks:
            out = []
            changed = False
            for inst in blk.instructions:
                si = inst.sync_info
                ow = list(si.on_wait) if (si is not None and si.on_wait) else []
                if len(ow) > max_waits:
                    for k, w in enumerate(ow[:-max_waits]):
                        out.append(mybir.InstNoOp(
                            name=f"{inst.name}_wsplit{k}",
                            engine=inst.engine,
                            ins=[], outs=[],
                            sync_info=mybir.SyncInfo(on_wait=[w],
                                                     on_update=[]),
                        ))
                    si.on_wait = ow[-max_waits:]
                    changed = True
                out.append(inst)
            if changed:
                blk.instructions = out
    return nc


def _features(pts):
    sx, sy = pts[:-1, 0], pts[:-1, 1]
    eX, eY = pts[1:, 0], pts[1:, 1]
    ex, ey = eX - sx, eY - sy
    c = ex * sy - ey * sx
    g0, g1, g2 = ex, -ey, -c
    one = np.ones(M)
    U6 = np.stack([g0 * g0, g1 * g1, g2 * g2,
                   g0 * g1, g0 * g2, g1 * g2], 0)
    V6 = np.stack([sy * eY, sx * eX, one,
                   sy * eX + sx * eY,
                   sy + eY,
                   sx + eX], 0)
    return U6, V6


def _host_prep(pp, ts, pm):
    import ml_dtypes
    bfd = ml_dtypes.bfloat16
    tc_cls = ts[:, :, 4].astype(np.int32)
    valid = ~pm
    nn = valid & (tc_cls != 0)

    per_core = []
    n_segs = []
    for b in range(B):
        order = np.argsort(~nn[b], kind="stable")
        pts = pp[b][order].astype(np.float64)
        n = int(nn[b].sum())
        n_seg = n - 1
        n_segs.append(n_seg)
        if n > 0:
            pts = pts - pts[:n].mean(axis=0)
        U6, V6 = _features(pts)
        one = np.ones(M)
        inv = (np.arange(M) >= max(n_seg, 0)).astype(np.float64) * BIG
        A12 = np.concatenate([V6, inv[None], one[None]], 0)
        B12 = np.concatenate([U6, one[None], inv[None]], 0)
        per_core.append(dict(n=n, n_seg=n_seg,
                             A12=A12.astype(bfd), B12=B12.astype(bfd),
                             A34=U6.astype(bfd), B34=V6.astype(bfd)))
    return tc_cls, valid, nn, per_core, n_segs


def _schedule(L, Jmax):
    fulls, tails = [], []
    for ib in range(L // 128):
        i0 = 128 * ib
        j0 = i0
        while j0 < Jmax:
            N = min(CHUNK, Jmax - j0)
            (fulls if N == CHUNK else tails).append((i0, j0, N))
            j0 += N
    if len(tails) > 1:
        tails = [tails[0]] + sorted(tails[1:], key=lambda c: -c[2])
    order = []
    fi = ti = 0
    while fi < len(fulls) or ti < len(tails):
        if fi < len(fulls):
            order.append(fulls[fi])
            fi += 1
        if ti < len(tails):
            order.append(tails[ti])
            ti += 1
    chunks = []
    pos = 0
    for (i0, j0, N) in order:
        chunks.append((i0, j0, N, pos))
        pos += N
    ops = []
    c = 0
    while c < len(chunks):
        k = 1 if c < 2 else min(2, len(chunks) - c)
        ops.append((c, k, chunks[c][3], sum(chunks[c + t][2] for t in range(k))))
        c += k
    opidx = {}
    for oi, (c0, k, _, _) in enumerate(ops):
        for t in range(k):
            opidx[c0 + t] = oi
    assert len(ops) <= NCOLS
    return chunks, ops, opidx, pos


def _build_program(L, Jmax):
    import concourse.bass as bass
    from concourse import mybir

    f32 = mybir.dt.float32
    bf16 = mybir.dt.bfloat16
    ALU = mybir.AluOpType
    ACT = mybir.ActivationFunctionType

    chunks, ops, opidx, total = _schedule(L, Jmax)
    SW = -(-total // 16) * 16
    NG = len(ops)

    nc = bass.Bass()
    d_feat = nc.dram_tensor("feat", [K12 + K34, 2 * L], bf16,
                            kind="ExternalInput")
    d_out = nc.dram_tensor("partials", [128, NG], f32, kind="ExternalOutput")

    with (
        nc.sbuf_tensor([38, 2 * L], bf16) as feat,
        nc.sbuf_tensor([128, SW], bf16) as sring,
        nc.sbuf_tensor([128, NCOLS], f32) as cols,
        nc.psum_tensor([128, 4096], f32) as qps,
        nc.semaphore("dma12_sem") as dma12_sem,
        nc.semaphore("dma34_sem") as dma34_sem,
        nc.semaphore("strt_sem") as strt_sem,
        nc.semaphore("pe12_sem") as pe12_sem,
        nc.semaphore("pe34_sem") as pe34_sem,
        nc.semaphore("act_sem") as act_sem,
        nc.semaphore("dve_sem") as dve_sem,
        nc.semaphore("out_sem") as out_sem,
        nc.Block() as block,
    ):
        fB12 = feat[0:K12, 0:L]
        fA12 = feat[0:K12, L:2 * L]
        fB34 = feat[32:32 + K34, 0:L]
        fA34 = feat[32:32 + K34, L:2 * L]
        JW = min(-(-Jmax // 8) * 8, L)

        @block.sync
        def _(sy):
            sy.dma_start(out=feat[0:K12, 0:JW],
                         in_=d_feat[0:K12, 0:JW]).then_inc(strt_sem, 16)
            sy.dma_start(out=feat[0:K12, L + 128:2 * L],
                         in_=d_feat[0:K12, L + 128:2 * L]).then_inc(dma12_sem, 16)

        @block.scalar
        def _(act):
            act.dma_start(out=feat[0:K12, L:L + 128],
                          in_=d_feat[0:K12, L:L + 128]).then_inc(strt_sem, 16)
            nc.scalar.activation(out=sring[:, 0:8], in_=sring[:, 0:8],
                                 func=ACT.Sigmoid, scale=0.0)
            for oi, (c0, k, pos0, W) in enumerate(ops):
                act.wait_ge(pe12_sem, oi + 1)
                o12 = 512 * (c0 % 4)
                nc.scalar.activation(out=sring[:, pos0:pos0 + W],
                                     in_=qps[:, o12:o12 + W],
                                     func=ACT.Sigmoid,
                                     scale=-0.01).then_inc(act_sem, 1)
            act.wait_ge(dve_sem, NG)
            act.dma_start(out=d_out[:, 0:NG],
                          in_=cols[:, 0:NG]).then_inc(out_sem, 16)

        @block.gpsimd
        def _(gp):
            gp.dma_start(out=feat[32:32 + K34, :],
                         in_=d_feat[K12:K12 + K34, :]).then_inc(dma34_sem, 16)

        @block.tensor
        def _(pe):
            nch = len(chunks)
            events = [("q12", 0), ("q12", 1), ("q34", 0), ("q34", 1)]
            for idx in range(2, nch + 2):
                if idx < nch:
                    events.append(("q12", idx))
                if idx - 2 >= 2:
                    events.append(("q34", idx - 2))
            first12 = True
            first34 = True
            for kind, c in events:
                oi = opidx[c]
                c0, k = ops[oi][0], ops[oi][1]
                t = c - c0
                i0, j0, N, _ = chunks[c]
                bank = c % 4
                if kind == "q12":
                    if first12:
                        pe.wait_ge(strt_sem, 32)
                        first12 = False
                    elif t == 0:
                        if c == 2:
                            pe.wait_ge(dma12_sem, 16)
                        if c >= 4:
                            pe.wait_ge(act_sem, opidx[c - 4] + 1)
                    mm = nc.tensor.matmul(
                        qps[:, 512 * bank:512 * bank + N],
                        fA12[:, i0:i0 + 128],
                        fB12[:, j0:j0 + N],
                        start=True, stop=True, tile_position=(0, 0))
                    if t == k - 1:
                        mm.then_inc(pe12_sem, 1)
                else:
                    if first34:
                        pe.wait_ge(dma34_sem, 16)
                        first34 = False
                    elif t == 0 and c >= 4:
                        pe.wait_ge(dve_sem, opidx[c - 4] + 1)
                    mm = nc.tensor.matmul(
                        qps[:, 2048 + 512 * bank:2048 + 512 * bank + N],
                        fA34[:, i0:i0 + 128],
                        fB34[:, j0:j0 + N],
                        start=True, stop=True, tile_position=(32, 0))
                    if t == k - 1:
                        mm.then_inc(pe34_sem, 1)

        @block.vector
        def _(dve):
            for oi, (c0, k, pos0, W) in enumerate(ops):
                dve.wait_ge(pe34_sem, oi + 1)
                dve.wait_ge(act_sem, oi + 1)
                o34 = 2048 + 512 * (c0 % 4)
                nc.vector.scalar_tensor_tensor(
                    out=sring[:, pos0:pos0 + W],
                    in0=qps[:, o34:o34 + W],
                    scalar=0.0, op0=ALU.is_lt,
                    in1=sring[:, pos0:pos0 + W], op1=ALU.mult,
                    accum_out=cols[:, oi:oi + 1]).then_inc(dve_sem, 1)


    _split_multi_waits(nc)
    return nc


def _sig(x):
    with np.errstate(over="ignore"):
        return 1.0 / (1.0 + np.exp(np.clip(x, -500.0, 500.0)))


def kernel(point_pred, orient_pred, class_pred, target_seq, padding_mask):
    pp = np.ascontiguousarray(np.asarray(point_pred, dtype=np.float32))
    op = np.ascontiguousarray(np.asarray(orient_pred, dtype=np.float32))
    cp = np.ascontiguousarray(np.asarray(class_pred, dtype=np.float32))
    ts = np.ascontiguousarray(np.asarray(target_seq, dtype=np.float32))
    pm = np.ascontiguousarray(np.asarray(padding_mask)).astype(bool)

    tc_cls, valid, nn, per_core, n_segs = _host_prep(pp, ts, pm)

    nsmax = max(max(n_segs), 1)
    L = max(128, -(-nsmax // 128) * 128)
    L = min(L, -(-M // 128) * 128)
    Jmax = min(-(-nsmax // 8) * 8, L)

    key = (L, Jmax)
    if key not in _CACHE:
        _CACHE[key] = _build_program(L, Jmax)
    nc = _CACHE[key]
    chunks, ops, _opidx, _total = _schedule(L, Jmax)
    NG = len(ops)

    import ml_dtypes
    bfdt = ml_dtypes.bfloat16
    in_maps = []
    for b in range(B):
        pc = per_core[b]
        featpk = np.zeros((K12 + K34, 2 * L), bfdt)
        w = min(M, L)
        featpk[0:K12, 0:w] = pc["B12"][:, :w]
        featpk[0:K12, L:L + w] = pc["A12"][:, :w]
        featpk[K12:K12 + K34, 0:w] = pc["B34"][:, :w]
        featpk[K12:K12 + K34, L:L + w] = pc["A34"][:, :w]
        if L > M:
            big_bf = bfdt(BIG)
            featpk[6, L + M:2 * L] = big_bf
            featpk[7, L + M:2 * L] = bfdt(1.0)
            featpk[6, M:L] = bfdt(1.0)
            featpk[7, M:L] = big_bf
        in_maps.append({"feat": np.ascontiguousarray(featpk)})

    from concourse.bass_utils import run_bass_kernel_spmd
    global LAST_RESULTS
    kw = dict(TRACE_KWARGS) if TRACE_KWARGS else {}
    res = run_bass_kernel_spmd(nc, in_maps, core_ids=list(range(NCORES)), **kw)
    LAST_RESULTS = res
    parts = [r["partials"] for r in res.results]

    f32 = np.float32

    isect_sum = np.float64(0.0)
    cnt_total = 0
    nb = L // 128
    for b in range(B):
        pc = per_core[b]
        n, n_seg = pc["n"], pc["n_seg"]
        raw = np.float64(parts[b][:, :NG].astype(np.float64).sum())
        A12 = pc["A12"].astype(np.float32)
        B12 = pc["B12"].astype(np.float32)
        A34 = pc["A34"].astype(np.float32)
        B34 = pc["B34"].astype(np.float32)
        junk = np.float64(0.0)
        for ib in range(nb):
            i0 = 128 * ib
            jw = min(DIAG_W, Jmax - i0)
            if jw <= 0:
                continue
            ia, ib_ = i0, min(i0 + 128, M)
            ja, jb_ = i0, min(i0 + jw, M)
            q12 = A12[:, ia:ib_].T @ B12[:, ja:jb_]
            q34 = A34[:, ia:ib_].T @ B34[:, ja:jb_]
            di = np.arange(ib_ - ia)[:, None]
            dj = np.arange(jb_ - ja)[None, :]
            msk = dj < di + 2
            s = _sig(0.01 * q12.astype(np.float64)) * (q34 < 0.0)
            junk += np.where(msk, s, 0.0).sum()
        wrap = np.float64(0.0)
        if n >= 4:
            jw_ = n_seg - 1
            q12w = np.float64(A12[:, 0].astype(np.float64)
                              @ B12[:, jw_].astype(np.float64))
            q34w = np.float64(A34[:, 0].astype(np.float64)
                              @ B34[:, jw_].astype(np.float64))
            wrap = _sig(0.01 * q12w) * float(q34w < 0.0)
            cnt_total += (n_seg - 1) * (n_seg - 2) // 2 - 1
            isect_sum += raw - junk - wrap

    if cnt_total > 0:
        isect_loss = f32(isect_sum / cnt_total)
    else:
        isect_loss = f32(0.0)

    valid_f = valid.astype(np.float64)
    nn_f = nn.astype(np.float64)
    vden = max(valid_f.sum(), 1.0)
    nden = max(nn_f.sum(), 1.0)

    x = cp.astype(np.float64)
    xmax = x.max(axis=-1, keepdims=True)
    lse = np.log(np.exp(x - xmax).sum(axis=-1)) + xmax[..., 0]
    sel = np.take_along_axis(x, tc_cls[..., None], axis=-1)[..., 0]
    cls_loss = f32(((lse - sel) * valid_f).sum() / vden)

    d = (pp.astype(np.float64) - ts[:, :, :2].astype(np.float64)) / RETINA
    ad = np.abs(d)
    sl1 = np.where(ad < 1.0, 0.5 * d * d, ad - 0.5).mean(axis=-1)
    pt_loss = f32((sl1 * nn_f).sum() / nden)

    cos = (op.astype(np.float64) * ts[:, :, 2:4].astype(np.float64)).sum(-1)
    orient_loss = f32(((1.0 - cos) * nn_f).sum() / nden)

    total = f32(pt_loss + f32(0.5) * orient_loss + cls_loss
                + f32(0.1) * isect_loss)
    return (total, pt_loss, orient_loss, cls_loss, isect_loss)
